# revision 2
# baseline (speedup 1.0000x reference)
"""Compact Bilinear Pooling (B=16, C=512, HW=196, OUT=8192) on 8 TRN2 cores.

Math (same reduction as baseline): with count-sketch (h_j, s_j),
    U_j[hw, f] = sum_c x[hw, c] * s_j[c] * w^(h_j[c] f),  w = e^{-2pi i/N}
    Q[f] = sum_hw U_1[hw, f] * U_2[hw, f]
    X = irfft(Q) * N, then signed-sqrt + L2 normalize.

This version generates the DFT-phase tables ON DEVICE (fp16, just-in-time
per 128-frequency chunk) from tiny integer seeds:
    m_im[c, f] = (h f + off) mod N,  m_re = m_im + 6144 mod N
    table = Sin(m * 2pi/N - pi)     [= -sin(theta) -> cos via +6144]
maintained incrementally across chunks (m += 128 h mod N). The matmul runs
table-stationary / x-moving, so U lands frequency-major and the spatial sum
is a free-dim reduce; Q is scattered into the [f1=64, f2=64] irfft grid by a
diagonalize+one-hot matmul. Epilogue (64x128 Cooley-Tukey irfft + tail) as
in the baseline.

x is pre-scaled by sqrt(1/8) so Q carries 1/8 (fp16 headroom in the grid
matmul); the epilogue multiplies by 16 (=2/alpha) and folds 8 into the
correction-row constants.

Sharding: data-parallel over batch, 2 batches per core, no collectives.
"""

import numpy as np

B, C, HW, N = 16, 512, 196, 8192
NCORES = 8
BPC = B // NCORES
NCHUNK = 32              # 32 chunks x 128 bins = f in [0, 4096); Nyquist extra
EPS_SQRT = 1e-5
EPS_NORM = 1e-12
ALPHA = 1.0
SX = float(np.sqrt(ALPHA))
W392 = BPC * HW          # 392 moving columns (hw x batch)

_COMPILED = {}
DBG = False


def _host_meta(sketch1, sketch2):
    """Extract (h, s) and build integer seed tables (host reformat only).

    Returns intpack [2(minit|hrep), 2(sk), 128, 4, 2, 128] int32 and
    vny [128, 2, 4] f32."""
    mset = []
    for sk in (sketch1, sketch2):
        sk = np.asarray(sk)
        h = np.abs(sk).argmax(axis=1).astype(np.int64)
        s = sk[np.arange(C), h]
        off = np.where(s < 0, N // 2, 0).astype(np.int64)
        mset.append((h, off, s))

    j = np.arange(128, dtype=np.int64)
    intpack = np.zeros((2, 2, 128, 4, 2, 128), np.int32)
    vny = np.zeros((128, 2, 4), np.float32)
    for i, (h, off, s) in enumerate(mset):
        hh = h.reshape(4, 128)           # [kc, p]
        oo = off.reshape(4, 128)
        base = hh[:, :, None] * j[None, None, :] + oo[:, :, None]  # [kc,p,j]
        m_im = (base) % N
        m_re = (base + 6144) % N
        intpack[0, i] = np.stack([m_im, m_re], axis=2).transpose(1, 0, 2, 3)
        hr = ((hh * 128) % N)[:, :, None, None]
        intpack[1, i] = np.broadcast_to(
            hr, (4, 128, 2, 128)).transpose(1, 0, 2, 3)
        vny[:, i, :] = (s.reshape(4, 128) * ((-1.0) ** (hh & 1))).T
    return intpack, vny


def _host_consts(vny):
    """Pack all f32 constants (and vny) into the const part of the f32
    image; the x block (cols [0, XCOLS)) is filled per core later."""
    sel = np.zeros((128, 128), np.float32)
    p = np.arange(128)
    sel[p, 63 + (p >= 64)] = 1.0
    band = np.zeros((128, 64), np.float32)
    band[p, p % 64] = 1.0

    k1 = np.arange(128)
    f2 = np.arange(64)
    k2 = np.arange(64)
    f1h = np.arange(64)
    E128c = np.cos(2 * np.pi * np.outer(f1h, k1) / 128).astype(np.float32)
    E128s = np.sin(2 * np.pi * np.outer(f1h, k1) / 128).astype(np.float32)
    TWc = np.cos(2 * np.pi * np.outer(f2, k1) / N).astype(np.float32)
    TWs = np.sin(2 * np.pi * np.outer(f2, k1) / N).astype(np.float32)
    E64c = np.cos(2 * np.pi * np.outer(f2, k2) / 64).astype(np.float32)
    E64s = np.sin(2 * np.pi * np.outer(f2, k2) / 64).astype(np.float32)
    inv_a = 1.0 / ALPHA
    cvals = {
        "sel": sel, "band": band, "vny": vny.reshape(128, 8),
        "e128c": E128c, "e128s": E128s, "e128sn": -E128s,
        "twc": TWc, "tws": TWs,
        "e64c": E64c, "e64sn": -E64s,
        "ones_col": np.ones((128, 1), np.float32),
        "ones_row": np.ones((1, 128), np.float32),
        "mones8_row": np.full((1, 128), -inv_a, np.float32),
        "alt8_row": (inv_a * (-1.0) ** np.arange(128)).reshape(1, 128)
        .astype(np.float32),
    }
    fpack = np.zeros((128, FW), np.float32)
    for k, shp in _CONST_SPECS:
        if k not in cvals:      # mi*/hr* blocks are filled by make_in_maps
            continue
        c0, w = _FOFF[k]
        fpack[:shp[0], c0:c0 + w] = cvals[k]
    return fpack


_CONST_SPECS = (
    ("sel", [128, 128]), ("band", [128, 64]), ("vny", [128, 8]),
    ("e128c", [64, 128]), ("e128s", [64, 128]), ("e128sn", [64, 128]),
    ("twc", [64, 128]), ("tws", [64, 128]),
    ("e64c", [64, 64]), ("e64sn", [64, 64]),
    ("ones_col", [128, 1]), ("ones_row", [1, 128]),
    ("mones8_row", [1, 128]), ("alt8_row", [1, 128]),
    # integer seed tables carried as exact f32 values (all < 16384)
    ("mi0", [128, 1024]), ("mi1", [128, 1024]),
    ("hr0", [128, 1024]), ("hr1", [128, 1024]),
)
XCOLS = 4 * BPC * HW                      # x block: [128, (kc, b, hw)]


def _flayout():
    """Column layout of the packed f32 input image [128, FW]."""
    off = {"x": (0, XCOLS)}
    c = XCOLS
    for k, shp in _CONST_SPECS:
        w = shp[1]
        off[k] = (c, w)
        c += w
    return off, c


_FOFF, FW = _flayout()


def _build_program():
    import concourse.bass as bass
    import concourse.mybir as mybir
    import concourse.tile as tile
    from concourse import bacc

    f32 = mybir.dt.float32
    f16 = mybir.dt.float16
    i32 = mybir.dt.int32
    AF = mybir.ActivationFunctionType
    OP = mybir.AluOpType
    X = mybir.AxisListType.X
    dtmap = {"f32": f32, "f16": f16}

    nc = bacc.Bacc("TRN2", target_bir_lowering=False, debug=False,
                   num_devices=NCORES)

    fpack_d = nc.dram_tensor("fpack", [128, FW], f32,
                             kind="ExternalInput").ap()
    out = nc.dram_tensor("out", [BPC, 128, 64], f32, kind="ExternalOutput").ap()
    dbg = {}
    if DBG:
        for k, shp, dt in (
            ("dbg_tab0", [128, 4, 2, 128], "f32"),
            ("dbg_tab1", [128, 4, 2, 128], "f32"),
            ("dbg_u00", [128, 2, 196], "f32"),
            ("dbg_q4_0", [128, 4], "f32"),
            ("dbg_q4_1", [128, 4], "f32"),
            ("dbg_G", [64, 4, 64], "f32"),
            ("dbg_qn", [1, 2], "f32"),
            ("dbg_x16", [128, 392], "f32"),
        ):
            dbg[k] = nc.dram_tensor(k, shp, dtmap[dt] if dt == "f32" else
                                    (f16 if dt == "f16" else f32),
                                    kind="ExternalOutput").ap()

    SCALE_SIN = float(2.0 * np.pi / N)
    NPI = -float(np.pi)

    with tile.TileContext(nc) as tc:
        with (
            tc.tile_pool(name="cpool", bufs=1) as cpool,
            tc.tile_pool(name="xpool", bufs=1) as xpool,
            tc.tile_pool(name="mpool", bufs=1) as mpool,
            tc.tile_pool(name="fpool", bufs=3) as fpool,
            tc.tile_pool(name="tpool", bufs=3) as tpool,
            tc.tile_pool(name="hpool", bufs=3) as hpool,
            tc.tile_pool(name="small", bufs=2) as small,
            tc.tile_pool(name="upsum", bufs=1, space="PSUM") as upsum,
            tc.tile_pool(name="gpsum", bufs=2, space="PSUM") as gpsum,
            tc.tile_pool(name="npsum", bufs=1, space="PSUM") as npsum,
            tc.tile_pool(name="spsum", bufs=1, space="PSUM") as spsum,
        ):
            # ---- constants (from packed f32 image) ----
            ct = {}
            for k, shp in _CONST_SPECS:
                t = cpool.tile(list(shp), f32, tag=k, name=k)
                c0, w = _FOFF[k]
                nc.sync.dma_start(t[:], fpack_d[0:shp[0], c0:c0 + w])
                ct[k] = t
            eps_b = cpool.tile([128, 1], f32, tag="eps_b", name="eps_b")
            nc.gpsimd.memset(eps_b[:], EPS_SQRT)
            eps_n = cpool.tile([128, 1], f32, tag="eps_n", name="eps_n")
            nc.gpsimd.memset(eps_n[:], float(N) * EPS_SQRT)
            npi = cpool.tile([128, 1], f32, tag="npi", name="npi")
            nc.gpsimd.memset(npi[:], NPI)

            # ---- x (f32), [128, b*hw] per kc ----
            x16 = []
            for kc in range(4):
                xt = xpool.tile([128, BPC * HW], f32, tag=f"xf_{kc}")
                nc.sync.dma_start(
                    xt[:], fpack_d[:, kc * BPC * HW:(kc + 1) * BPC * HW])
                x16.append(xt)

            # ---- seed tables: f32-valued ints -> int32 tiles ----
            m_t, hr_t = [], []
            for sk in range(2):
                mt = mpool.tile([128, 4, 2, 128], i32, tag=f"m{sk}")
                nc.vector.tensor_copy(
                    mt[:], ct[f"mi{sk}"][:].rearrange(
                        "p (a b c) -> p a b c", a=4, b=2, c=128))
                m_t.append(mt)
                ht = mpool.tile([128, 4, 2, 128], i32, tag=f"h{sk}")
                nc.vector.tensor_copy(
                    ht[:], ct[f"hr{sk}"][:].rearrange(
                        "p (a b c) -> p a b c", a=4, b=2, c=128))
                hr_t.append(ht)
            vt = ct["vny"]

            # ---- Nyquist bin f=4096: U_j = x @ v_j, Qny = sum_hw U1*U2 ----
            un = [upsum.tile([128, BPC, HW], f32, tag=f"u{sk}0",
                             name=f"un{sk}")
                  for sk in range(2)]
            for sk in range(2):
                for kc in range(4):
                    nc.tensor.matmul(un[sk][0:1, :, :],
                                     vt[:, sk * 4 + kc:sk * 4 + kc + 1],
                                     x16[kc][:], start=(kc == 0),
                                     stop=(kc == 3))
            u2n = small.tile([1, BPC, HW], f32, tag="u2n")
            nc.scalar.copy(u2n[:], un[1][0:1, :, :])
            hn = small.tile([1, BPC, HW], f32, tag="hn")
            nc.vector.tensor_tensor(hn[:], un[0][0:1, :, :], u2n[:],
                                    op=OP.mult)
            qn = small.tile([1, BPC], f32, tag="qn")
            nc.vector.reduce_sum(qn[:], hn[:], axis=X)

            # ---- main loop over 32 frequency chunks ----
            # G accumulates in SBUF; each chunk's scatter matmuls are
            # single-shot PSUM groups added in by DVE.
            G = cpool.tile([64, 4, 64], f32, tag="G", name="G")
            nc.gpsimd.memset(G[:], 0.0)
            for c in range(NCHUNK):
                tab = []
                for sk in range(2):
                    if c > 0:
                        nc.vector.tensor_tensor(m_t[sk][:], m_t[sk][:],
                                                hr_t[sk][:], op=OP.add)
                    ta = fpool.tile([128, 4, 2, 128], i32, tag=f"ta{sk}")
                    nc.vector.tensor_scalar(ta[:], m_t[sk][:], 8191, None,
                                            op0=OP.bitwise_and)
                    tb = tpool.tile([128, 4, 2, 128], f32, tag=f"tab{sk}")
                    nc.scalar.activation(tb[:], ta[:], AF.Sin,
                                         scale=SCALE_SIN, bias=npi[:, 0:1])
                    tab.append(tb)

                # U accumulation: sketch1 first (copied out), sketch0 last
                U = [[None, None], [None, None]]
                for sk, part in ((1, 0), (1, 1), (0, 0), (0, 1)):
                    u = upsum.tile([128, BPC, HW], f32, tag=f"u{sk}{part}")
                    for kc in range(4):
                        nc.tensor.matmul(u[:], tab[sk][:, kc, part, :],
                                         x16[kc][:], start=(kc == 0),
                                         stop=(kc == 3))
                    U[sk][part] = u

                u2i = hpool.tile([128, BPC, HW], f32, tag="u2i")
                nc.scalar.copy(u2i[:], U[1][0][:])
                u2r = hpool.tile([128, BPC, HW], f32, tag="u2r")
                nc.scalar.copy(u2r[:], U[1][1][:])

                t_ii = hpool.tile([128, BPC, HW], f32, tag="t_ii")
                nc.vector.tensor_tensor(t_ii[:], U[0][0][:], u2i[:],
                                        op=OP.mult)
                t_ir = hpool.tile([128, BPC, HW], f32, tag="t_ir")
                nc.vector.tensor_tensor(t_ir[:], U[0][0][:], u2r[:],
                                        op=OP.mult)
                t_rr = hpool.tile([128, BPC, HW], f32, tag="t_rr")
                nc.vector.tensor_tensor(t_rr[:], U[0][1][:], u2r[:],
                                        op=OP.mult)
                t_ri = hpool.tile([128, BPC, HW], f32, tag="t_ri")
                nc.vector.tensor_tensor(t_ri[:], U[0][1][:], u2i[:],
                                        op=OP.mult)
                Hr = hpool.tile([128, BPC, HW], f32, tag="Hr")
                nc.gpsimd.tensor_tensor(Hr[:], t_rr[:], t_ii[:],
                                        op=OP.subtract)
                Hi = hpool.tile([128, BPC, HW], f32, tag="Hi")
                nc.gpsimd.tensor_tensor(Hi[:], t_ri[:], t_ir[:], op=OP.add)

                # Q4 cols: (re b0, re b1, im b0, im b1); note Hr is [128,b,hw]
                q4 = hpool.tile([128, 4], f32, tag="q4")
                nc.vector.reduce_sum(q4[:, 0:2], Hr[:], axis=X)
                nc.vector.reduce_sum(q4[:, 2:4], Hi[:], axis=X)
                if DBG and c in (0, 1):
                    nc.sync.dma_start(dbg[f"dbg_q4_{c}"][:], q4[:])
                if DBG and c == 0:
                    u00 = hpool.tile([128, 2, 196], f32, tag="dbg_u00")
                    nc.scalar.copy(u00[:], U[0][0][:])
                    nc.sync.dma_start(dbg["dbg_u00"][:], u00[:])

                # scatter into grid: Gc[f1, 2b+part, f2], then G += Gc (SBUF)
                gc = gpsum.tile([64, 4, 64], f32, tag="gc")
                for b in range(BPC):
                    for part in range(2):  # 0=re, 1=im
                        d = hpool.tile([128, 64], f32, tag=f"d{b}{part}")
                        nc.scalar.mul(d[:], ct["band"][:],
                                      q4[:, 2 * part + b:2 * part + b + 1])
                        nc.tensor.matmul(gc[:, 2 * b + part, :],
                                         ct["sel"][:, 63 - 2 * c:127 - 2 * c],
                                         d[:], start=True, stop=True)
                nc.vector.tensor_tensor(G[:], G[:], gc[:], op=OP.add)

            if DBG:
                gsb = small.tile([64, 4, 64], f32, tag="dbg_gsb")
                nc.scalar.copy(gsb[:], G[:])
                nc.sync.dma_start(dbg["dbg_G"][:], gsb[:])
                nc.sync.dma_start(dbg["dbg_qn"][:], qn[:])
                nc.sync.dma_start(dbg["dbg_x16"][:], x16[0][:])

            # ---- per batch: irfft grid -> output tail ----
            for b in range(BPC):
                xr = small.tile([64, 64], f32, tag="xr")
                xi = small.tile([64, 64], f32, tag="xi")
                nc.scalar.copy(xr[:], G[:, 2 * b, :])
                nc.scalar.copy(xi[:], G[:, 2 * b + 1, :])

                sps = spsum.tile([128, 512], f32, tag="sps")
                yr = sps[0:64, 0:128]
                yi = sps[0:64, 128:256]
                zps = sps[0:128, 256:320]
                tot = sps[0:1, 320:321]
                nrmb = sps[0:128, 352:353]
                cps = sps[0:128, 384:385]

                # c[k1] = (-Q0 + (-1)^k1 Qny) / alpha
                nc.tensor.matmul(cps, ct["mones8_row"][:], xr[0:1, 0:1],
                                 start=True, stop=False)
                nc.tensor.matmul(cps, ct["alt8_row"][:], qn[0:1, b:b + 1],
                                 start=False, stop=True)
                c_sb = small.tile([128, 1], f32, tag="c_sb")
                nc.scalar.copy(c_sb[:], cps)

                # stage 1: Y[f2, k1] = sum_f1 G[f1, f2] e^{+2pi i k1 f1/128}
                nc.tensor.matmul(yr, xr[:], ct["e128c"][:], start=True,
                                 stop=False)
                nc.tensor.matmul(yr, xi[:], ct["e128sn"][:], start=False,
                                 stop=True)
                nc.tensor.matmul(yi, xr[:], ct["e128s"][:], start=True,
                                 stop=False)
                nc.tensor.matmul(yi, xi[:], ct["e128c"][:], start=False,
                                 stop=True)

                # twiddle
                ypr = small.tile([64, 128], f32, tag="ypr")
                ypi = small.tile([64, 128], f32, tag="ypi")
                tt1 = small.tile([64, 128], f32, tag="tt1")
                tt2 = small.tile([64, 128], f32, tag="tt2")
                nc.vector.tensor_tensor(tt1[:], yr, ct["twc"][:], op=OP.mult)
                nc.vector.tensor_tensor(tt2[:], yi, ct["tws"][:], op=OP.mult)
                nc.vector.tensor_tensor(ypr[:], tt1[:], tt2[:],
                                        op=OP.subtract)
                nc.vector.tensor_tensor(tt1[:], yr, ct["tws"][:], op=OP.mult)
                nc.vector.tensor_tensor(tt2[:], yi, ct["twc"][:], op=OP.mult)
                nc.vector.tensor_tensor(ypi[:], tt1[:], tt2[:], op=OP.add)

                # stage 2
                nc.tensor.matmul(zps, ypr[:], ct["e64c"][:], start=True,
                                 stop=False)
                nc.tensor.matmul(zps, ypi[:], ct["e64sn"][:], start=False,
                                 stop=True)

                # Z = zps*2/alpha + c
                zeff = small.tile([128, 64], f32, tag="zeff")
                nc.vector.tensor_scalar(zeff[:], zps, float(2.0 / ALPHA),
                                        c_sb[:, 0:1], op0=OP.mult, op1=OP.add)

                # tail: signed sqrt + L2 normalize
                absz = small.tile([128, 64], f32, tag="absz")
                nc.scalar.activation(absz[:], zeff[:], AF.Abs)
                sq = small.tile([128, 64], f32, tag="sq")
                nc.scalar.activation(sq[:], absz[:], AF.Sqrt, bias=eps_b[:])
                sgn = small.tile([128, 64], f32, tag="sgn")
                nc.scalar.activation(sgn[:], zeff[:], AF.Sign)
                ssq = small.tile([128, 64], f32, tag="ssq")
                nc.vector.tensor_tensor(ssq[:], sq[:], sgn[:], op=OP.mult)
                rs = small.tile([128, 1], f32, tag="rs")
                nc.vector.reduce_sum(rs[:], zeff[:], axis=X,
                                     apply_absolute_value=True)
                nc.tensor.matmul(tot, rs[:], ct["ones_col"][:], start=True,
                                 stop=True)
                nrm = small.tile([1, 1], f32, tag="nrm")
                nc.scalar.activation(nrm[:], tot, AF.Sqrt, bias=eps_n[0:1, :])
                nc.vector.tensor_scalar_max(nrm[:], nrm[:], EPS_NORM)
                nc.vector.reciprocal(nrm[:], nrm[:])
                nc.tensor.matmul(nrmb, ct["ones_row"][:], nrm[:], start=True,
                                 stop=True)
                nrmb_s = small.tile([128, 1], f32, tag="nrmb_s")
                nc.scalar.copy(nrmb_s[:], nrmb)
                fin = small.tile([128, 64], f32, tag="fin")
                nc.vector.tensor_scalar_mul(fin[:], ssq[:], nrmb_s[:])
                nc.sync.dma_start(out[b], fin[:])

    nc.compile()
    return nc


def _get_program():
    if "nc" not in _COMPILED:
        _COMPILED["nc"] = _build_program()
    return _COMPILED["nc"]


def make_in_maps(x, sketch1, sketch2):
    x = np.ascontiguousarray(np.asarray(x), dtype=np.float32)
    intpack, vny = _host_meta(sketch1, sketch2)
    fpack0 = _host_consts(vny)
    for sk in range(2):
        c0, w = _FOFF[f"mi{sk}"]
        fpack0[:, c0:c0 + w] = intpack[0, sk].reshape(128, 1024)
        c0, w = _FOFF[f"hr{sk}"]
        fpack0[:, c0:c0 + w] = intpack[1, sk].reshape(128, 1024)
    xs = x.reshape(B, C, HW)
    in_maps = []
    for i in range(NCORES):
        fp = fpack0.copy()
        # x block: [128, (kc, b, hw)] = per-kc channel slices side by side
        xc = xs[i * BPC:(i + 1) * BPC]          # [BPC, C, HW]
        fp[:, 0:XCOLS] = (xc.reshape(BPC, 4, 128, HW)
                          .transpose(2, 1, 0, 3).reshape(128, XCOLS))
        in_maps.append({"fpack": fp})
    return in_maps


def unshard_out(results):
    outs = np.empty((B, N), dtype=np.float32)
    for i in range(NCORES):
        z = results[i]["out"]  # [BPC, 128, 64]
        for j in range(BPC):
            outs[i * BPC + j] = np.ascontiguousarray(z[j].T).reshape(-1)
    return outs


def kernel(x, sketch1, sketch2):
    from concourse.bass_utils import run_bass_kernel_spmd

    in_maps = make_in_maps(x, sketch1, sketch2)
    nc = _get_program()
    res = run_bass_kernel_spmd(nc, in_maps, core_ids=list(range(NCORES)))
    return unshard_out(res.results)
